# revision 8
# baseline (speedup 1.0000x reference)
"""MoE layer (top-2 of 8 experts) on 8 Trainium2 NeuronCores.

Strategy (expert-parallel, matching the sharding hint):
  - Host computes the gate (logits -> top-2 -> softmax) and the aux loss;
    this is 0.05% of the FLOPs.
  - Tokens are dispatched per expert on the host (the "all-to-all"), padded
    to a fixed capacity C, and core e runs expert e's FFN:
        y = gate * (GELU(x @ W1[e] + b1[e]) @ W2[e] + b2[e])
    on its block of routed tokens.  Both GEMMs run in bf16 with fp32 PSUM
    accumulation.  The first GEMM is computed transposed (A^T = W1^T @ x^T)
    so the second GEMM needs no on-device transpose.
  - Host scatter-adds the per-expert results back into token order.

Device time is dominated by the two GEMMs: 2 * C * (D*H + H*DOUT) flops
per core ~ 19 GFLOP, near the bf16 PE roofline.
"""

import numpy as np
import ml_dtypes

# ---- problem constants (hardcoded per contract) ----
B, S, D, DOUT, E, H, K = 2, 2048, 1024, 1024, 8, 4096, 2
N = B * S
AUX_COEFF = 0.01
P = 128
C = 1152            # per-expert token capacity per round (multiple of 128)
KD = D // P         # 8  contraction chunks, GEMM1
MH = H // P         # 32 output row tiles, GEMM1 (H on partitions)
KH = H // P         # 32 contraction chunks, GEMM2
CT = C // P         # 9  output row tiles, GEMM2 (tokens on partitions)
C_TILES = [(0, 512), (512, 512), (1024, C - 1024)]  # GEMM1 free-dim tiling

BF16 = ml_dtypes.bfloat16

_COMPILED = None  # cached (nc, run) so repeated kernel() calls reuse the NEFF
LAST_EXEC_NS = None   # max-core HW exec time from the last traced run
LAST_TRACE = None     # path to the last perfetto trace (trace runs only)


def _build_program(act="gelu"):
    import concourse.bass as bass
    import concourse.mybir as mybir
    import concourse.tile as tile
    from concourse import bacc

    act_fn = {
        "gelu": mybir.ActivationFunctionType.Gelu,
        # CoreSim has no Gelu LUT; identity lets the sim validate everything else
        "identity": mybir.ActivationFunctionType.Identity,
    }[act]

    f32 = mybir.dt.float32
    bf16 = mybir.dt.bfloat16

    nc = bacc.Bacc(
        "TRN2",
        target_bir_lowering=False,
        debug=False,
        enable_asserts=True,
        num_devices=8,
    )

    # Inputs are pre-laid-out on the host so every DMA is contiguous:
    #   xt : [P, KD, C]  bf16   xt[p,k,c]  = x_routed[c, k*P+p]
    #   w1 : [MH, P, KD, P] bf16 w1[m,p,k,j] = W1[k*P+p, m*P+j]
    #   w2 : [P, KH, DOUT] bf16 w2[p,k,n]  = W2[k*P+p, n]
    #   b1 : [P, MH] f32        b1[p,m]    = b1[m*P+p]
    #   b2 : [P, DOUT] f32      row-broadcast copy of b2
    #   gv : [P, CT] f32        gv[p,t]    = gate[t*P+p]
    xt_d = nc.dram_tensor("xt", [P, KD, C], bf16, kind="ExternalInput").ap()
    w1_d = nc.dram_tensor("w1", [MH, P, KD, P], bf16, kind="ExternalInput").ap()
    w2_d = nc.dram_tensor("w2", [P, KH, DOUT], bf16, kind="ExternalInput").ap()
    b1_d = nc.dram_tensor("b1v", [P, MH], f32, kind="ExternalInput").ap()
    b2_d = nc.dram_tensor("b2v", [P, DOUT], f32, kind="ExternalInput").ap()
    g_d = nc.dram_tensor("gv", [P, CT], f32, kind="ExternalInput").ap()
    y_d = nc.dram_tensor("y", [CT, P, DOUT], f32, kind="ExternalOutput").ap()

    with tile.TileContext(nc) as tc:
        with (
            tc.tile_pool(name="resident", bufs=1) as res,
            tc.tile_pool(name="w1s", bufs=3) as w1s,
            tc.tile_pool(name="yout", bufs=4) as yout,
            tc.tile_pool(name="ps", bufs=6, space="PSUM") as psp,
        ):
            xt_sb = res.tile([P, KD, C], bf16)
            nc.sync.dma_start(xt_sb[:], xt_d[:])
            w2_sb = res.tile([P, KH, DOUT], bf16)
            nc.sync.dma_start(w2_sb[:], w2_d[:])
            b1_sb = res.tile([P, MH], f32)
            nc.sync.dma_start(b1_sb[:], b1_d[:])
            b2_sb = res.tile([P, DOUT], f32)
            nc.sync.dma_start(b2_sb[:], b2_d[:])
            g_sb = res.tile([P, CT], f32)
            nc.sync.dma_start(g_sb[:], g_d[:])
            hmt_sb = res.tile([P, KH, C], bf16)

            # GEMM1: hmt[:, m, :] = GELU(W1_m^T @ x^T + b1_m), H on partitions
            for m in range(MH):
                w1_t = w1s.tile([P, KD, P], bf16, tag="w1t")
                nc.sync.dma_start(w1_t[:], w1_d[m])
                ps_tiles = []
                for noff, nsz in C_TILES:
                    ps_tiles.append(psp.tile([P, nsz], f32, tag="ps", name=f"ps1_{m}_{noff}"))
                for k in range(KD):
                    for (noff, nsz), ps in zip(C_TILES, ps_tiles):
                        nc.tensor.matmul(
                            ps[:],
                            lhsT=w1_t[:, k],
                            rhs=xt_sb[:, k, noff : noff + nsz],
                            start=(k == 0),
                            stop=(k == KD - 1),
                        )
                for (noff, nsz), ps in zip(C_TILES, ps_tiles):
                    nc.scalar.activation(
                        hmt_sb[:, m, noff : noff + nsz],
                        ps[:],
                        act_fn,
                        bias=b1_sb[:, m : m + 1],
                        scale=1.0,
                    )

            # GEMM2: y[mc] = gate * (Hm_mc @ W2 + b2), tokens on partitions
            for mc in range(CT):
                ps0 = psp.tile([P, 512], f32, tag="ps", name=f"ps2a_{mc}")
                ps1 = psp.tile([P, 512], f32, tag="ps", name=f"ps2b_{mc}")
                for k in range(KH):
                    lhs = hmt_sb[:, k, mc * P : (mc + 1) * P]
                    nc.tensor.matmul(
                        ps0[:], lhsT=lhs, rhs=w2_sb[:, k, 0:512],
                        start=(k == 0), stop=(k == KH - 1),
                    )
                    nc.tensor.matmul(
                        ps1[:], lhsT=lhs, rhs=w2_sb[:, k, 512:1024],
                        start=(k == 0), stop=(k == KH - 1),
                    )
                y_t = yout.tile([P, DOUT], f32, tag="yt")
                nc.vector.tensor_add(y_t[:, 0:512], ps0[:], b2_sb[:, 0:512])
                nc.vector.tensor_add(y_t[:, 512:1024], ps1[:], b2_sb[:, 512:1024])
                nc.scalar.mul(y_t[:], y_t[:], g_sb[:, mc : mc + 1])
                nc.sync.dma_start(y_d[mc], y_t[:])

    nc.compile()
    return nc


def _get_program():
    global _COMPILED
    if _COMPILED is None:
        _COMPILED = _build_program()
    return _COMPILED


def _route(x_flat, Wg, bg):
    """Host gating: logits, top-2 (matches jax.lax.top_k tie-breaking),
    softmax gates, aux loss."""
    logits = x_flat.astype(np.float32) @ Wg.astype(np.float32) + bg.astype(np.float32)
    order = np.argsort(-logits, axis=-1, kind="stable")[:, :K]       # [N, K]
    topv = np.take_along_axis(logits, order, axis=1)                 # [N, K]
    mx = topv.max(axis=1, keepdims=True)
    eg = np.exp(topv - mx)
    gates = eg / eg.sum(axis=1, keepdims=True)                       # [N, K]

    lmx = logits.max(axis=1, keepdims=True)
    lse = np.log(np.exp(logits - lmx).sum(axis=1, keepdims=True)) + lmx
    log_probs = logits - lse
    ideal = 1.0 / E
    aux = AUX_COEFF * np.mean(ideal * (np.log(ideal) - log_probs), dtype=np.float64)
    return order, gates, np.float32(aux)


def kernel(x, Wg, bg, W1, b1, W2, b2):
    from concourse.bass_utils import run_bass_kernel_spmd

    x = np.asarray(x)
    x_flat = np.ascontiguousarray(x.reshape(N, D), dtype=np.float32)
    Wg, bg = np.asarray(Wg), np.asarray(bg)
    W1, b1, W2, b2 = (np.asarray(a, dtype=np.float32) for a in (W1, b1, W2, b2))

    order, gates, aux = _route(x_flat, Wg, bg)

    # per-expert token index lists and combined gate weights
    idx_e, g_e = [], []
    for e in range(E):
        hits = order == e                           # [N, K]
        w = (gates * hits).sum(axis=1).astype(np.float32)
        idx = np.nonzero(hits.any(axis=1))[0]
        idx_e.append(idx)
        g_e.append(w[idx])
    max_cnt = max(len(i) for i in idx_e)
    rounds = max(1, -(-max_cnt // C))

    # static per-core tensors (weights), laid out for contiguous DMA
    x_bf = x_flat.astype(BF16)
    static_maps = []
    for e in range(E):
        w1h = np.ascontiguousarray(
            W1[e].astype(BF16).reshape(KD, P, MH, P).transpose(2, 1, 0, 3)
        )                                            # [MH, P, KD, P]
        w2h = np.ascontiguousarray(
            W2[e].astype(BF16).reshape(KH, P, DOUT).transpose(1, 0, 2)
        )                                            # [P, KH, DOUT]
        b1h = np.ascontiguousarray(b1[e].reshape(MH, P).T)           # [P, MH]
        b2h = np.ascontiguousarray(np.broadcast_to(b2[e], (P, DOUT)))  # [P, DOUT]
        static_maps.append({"w1": w1h, "w2": w2h, "b1v": b1h, "b2v": b2h})

    nc = _get_program()

    out_flat = np.zeros((N, DOUT), dtype=np.float32)
    for r in range(rounds):
        in_maps = []
        chunk_idx = []
        for e in range(E):
            idx = idx_e[e][r * C : (r + 1) * C]
            g = g_e[e][r * C : (r + 1) * C]
            chunk_idx.append(idx)
            xg = np.zeros((C, D), dtype=BF16)
            xg[: len(idx)] = x_bf[idx]
            xth = np.ascontiguousarray(
                xg.reshape(C, KD, P).transpose(2, 1, 0)
            )                                        # [P, KD, C]
            gp = np.zeros(C, dtype=np.float32)
            gp[: len(g)] = g
            gh = np.ascontiguousarray(gp.reshape(CT, P).T)           # [P, CT]
            in_maps.append({**static_maps[e], "xt": xth, "gv": gh})

        import os

        global LAST_EXEC_NS, LAST_TRACE
        trace = bool(int(os.environ.get("KERNEL_TRACE", "0")))
        res = run_bass_kernel_spmd(nc, in_maps, list(range(E)), trace=trace)
        if trace:
            LAST_EXEC_NS = res.exec_time_ns
            if res.instructions_and_trace is not None:
                LAST_TRACE = res.instructions_and_trace[1]
        results = res.results

        for e in range(E):
            idx = chunk_idx[e]
            y = np.asarray(results[e]["y"]).reshape(C, DOUT)
            # idx is unique within an expert, so fancy-index += is safe
            out_flat[idx] += y[: len(idx)]

    return out_flat.reshape(B, S, DOUT), aux


# revision 13
# speedup vs baseline: 1.0074x; 1.0074x over previous
"""MoE layer (top-2 of 8 experts) on 8 Trainium2 NeuronCores.

Strategy (expert-parallel, matching the sharding hint):
  - Host computes the gate (logits -> top-2 -> softmax) and the aux loss;
    this is 0.05% of the FLOPs.
  - Tokens are dispatched per expert on the host (the "all-to-all"), padded
    to a fixed capacity C, and core e runs expert e's FFN:
        y = gate * (GELU(x @ W1[e] + b1[e]) @ W2[e] + b2[e])
    on its block of routed tokens.  Both GEMMs run in bf16 with fp32 PSUM
    accumulation.  The first GEMM is computed transposed (A^T = W1^T @ x^T)
    so the second GEMM needs no on-device transpose.
  - Host scatter-adds the per-expert results back into token order.

Device time is dominated by the two GEMMs: 2 * C * (D*H + H*DOUT) flops
per core ~ 19 GFLOP, near the bf16 PE roofline.
"""

import numpy as np
import ml_dtypes

# ---- problem constants (hardcoded per contract) ----
B, S, D, DOUT, E, H, K = 2, 2048, 1024, 1024, 8, 4096, 2
N = B * S
AUX_COEFF = 0.01
P = 128
C = 1088            # per-expert token capacity per round (max count is 1066)
C_PAD = 1152        # C rounded up to a multiple of 128 (DRAM layout granularity)
KD = D // P         # 8  contraction chunks, GEMM1
MH = H // P         # 32 output row tiles, GEMM1 (H on partitions)
KH = H // P         # 32 contraction chunks, GEMM2
CT = C_PAD // P     # 9  output row tiles, GEMM2 (tokens on partitions)
C_TILES = [(0, 512), (512, 512), (1024, C - 1024)]   # GEMM1 free-dim tiling
M_TILES = [(i * P, min(P, C - i * P)) for i in range(CT) if i * P < C]

BF16 = ml_dtypes.bfloat16

_COMPILED = None  # cached (nc, run) so repeated kernel() calls reuse the NEFF
LAST_EXEC_NS = None   # max-core HW exec time from the last traced run
LAST_TRACE = None     # path to the last perfetto trace (trace runs only)


def _build_program(act="gelu"):
    import concourse.bass as bass
    import concourse.mybir as mybir
    import concourse.tile as tile
    from concourse import bacc

    act_fn = {
        "gelu": mybir.ActivationFunctionType.Gelu,
        # CoreSim has no Gelu LUT; identity lets the sim validate everything else
        "identity": mybir.ActivationFunctionType.Identity,
    }[act]

    f32 = mybir.dt.float32
    bf16 = mybir.dt.bfloat16

    nc = bacc.Bacc(
        "TRN2",
        target_bir_lowering=False,
        debug=False,
        enable_asserts=True,
        num_devices=8,
    )

    # Inputs are pre-laid-out on the host so every DMA is contiguous:
    #   xt : [P, KD, C]  bf16   xt[p,k,c]  = x_routed[c, k*P+p]
    #   w1 : [MH, P, KD, P] bf16 w1[m,p,k,j] = W1[k*P+p, m*P+j]
    #   w2 : [P, KH, DOUT] bf16 w2[p,k,n]  = W2[k*P+p, n]
    #   b1 : [P, MH] f32        b1[p,m]    = b1[m*P+p]
    #   b2 : [P, DOUT] f32      row-broadcast copy of b2
    #   gv : [P, CT] f32        gv[p,t]    = gate[t*P+p]
    xt_d = nc.dram_tensor("xt", [P, KD, C], bf16, kind="ExternalInput").ap()
    w1_d = nc.dram_tensor("w1", [MH, P, KD, P], bf16, kind="ExternalInput").ap()
    w2_d = nc.dram_tensor("w2", [P, KH, DOUT], bf16, kind="ExternalInput").ap()
    b1_d = nc.dram_tensor("b1v", [P, MH], f32, kind="ExternalInput").ap()
    b2_d = nc.dram_tensor("b2v", [P, DOUT], f32, kind="ExternalInput").ap()
    g_d = nc.dram_tensor("gv", [P, CT], f32, kind="ExternalInput").ap()
    y_d = nc.dram_tensor("y", [CT, P, DOUT], f32, kind="ExternalOutput").ap()

    with tile.TileContext(nc) as tc:
        with (
            tc.tile_pool(name="resident", bufs=1) as res,
            tc.tile_pool(name="w1s", bufs=3) as w1s,
            tc.tile_pool(name="yout", bufs=4) as yout,
            tc.tile_pool(name="ps", bufs=8, space="PSUM") as psp,
        ):
            # xt loaded per k-chunk so the first matmul starts after ~1 chunk;
            # w2/b2 go on the (otherwise idle) GpSimd DMA queue so the big W2
            # transfer doesn't block the W1 stream on the Sync queue.
            xt_sb = res.tile([P, KD, C], bf16)
            for k in range(KD):
                nc.sync.dma_start(xt_sb[:, k], xt_d[:, k])
            b1_sb = res.tile([P, MH], f32)
            nc.sync.dma_start(b1_sb[:], b1_d[:])
            g_sb = res.tile([P, CT], f32)
            nc.sync.dma_start(g_sb[:], g_d[:])
            w2_sb = res.tile([P, KH, DOUT], bf16)
            nc.gpsimd.dma_start(w2_sb[:], w2_d[:])
            b2_sb = res.tile([P, DOUT], f32)
            nc.gpsimd.dma_start(b2_sb[:], b2_d[:])
            hmt_sb = res.tile([P, KH, C], bf16)

            # GEMM1: hmt[:, m, :] = GELU(W1_m^T @ x^T + b1_m), H on partitions
            for m in range(MH):
                w1_t = w1s.tile([P, KD, P], bf16, tag="w1t")
                nc.sync.dma_start(w1_t[:], w1_d[m])
                ps_tiles = []
                for noff, nsz in C_TILES:
                    ps_tiles.append(psp.tile([P, nsz], f32, tag="ps", name=f"ps1_{m}_{noff}"))
                for k in range(KD):
                    for (noff, nsz), ps in zip(C_TILES, ps_tiles):
                        nc.tensor.matmul(
                            ps[:],
                            lhsT=w1_t[:, k],
                            rhs=xt_sb[:, k, noff : noff + nsz],
                            start=(k == 0),
                            stop=(k == KD - 1),
                        )
                for (noff, nsz), ps in zip(C_TILES, ps_tiles):
                    nc.scalar.activation(
                        hmt_sb[:, m, noff : noff + nsz],
                        ps[:],
                        act_fn,
                        bias=b1_sb[:, m : m + 1],
                        scale=1.0,
                    )

            # GEMM2: y[mc] = gate * (Hm_mc @ W2 + b2), tokens on partitions
            for mc, (roff, rows) in enumerate(M_TILES):
                ps0 = psp.tile([P, 512], f32, tag="ps", name=f"ps2a_{mc}")
                ps1 = psp.tile([P, 512], f32, tag="ps", name=f"ps2b_{mc}")
                for k in range(KH):
                    lhs = hmt_sb[:, k, roff : roff + rows]
                    nc.tensor.matmul(
                        ps0[:rows], lhsT=lhs, rhs=w2_sb[:, k, 0:512],
                        start=(k == 0), stop=(k == KH - 1),
                    )
                    nc.tensor.matmul(
                        ps1[:rows], lhsT=lhs, rhs=w2_sb[:, k, 512:1024],
                        start=(k == 0), stop=(k == KH - 1),
                    )
                y_t = yout.tile([P, DOUT], f32, tag="yt")
                nc.vector.tensor_add(y_t[:rows, 0:512], ps0[:rows], b2_sb[:rows, 0:512])
                nc.vector.tensor_add(y_t[:rows, 512:1024], ps1[:rows], b2_sb[:rows, 512:1024])
                nc.scalar.mul(y_t[:rows], y_t[:rows], g_sb[:rows, mc : mc + 1])
                nc.sync.dma_start(y_d[mc, :rows], y_t[:rows])

    nc.compile()
    return nc


def _get_program():
    global _COMPILED
    if _COMPILED is None:
        _COMPILED = _build_program()
    return _COMPILED


def _route(x_flat, Wg, bg):
    """Host gating: logits, top-2 (matches jax.lax.top_k tie-breaking),
    softmax gates, aux loss."""
    logits = x_flat.astype(np.float32) @ Wg.astype(np.float32) + bg.astype(np.float32)
    order = np.argsort(-logits, axis=-1, kind="stable")[:, :K]       # [N, K]
    topv = np.take_along_axis(logits, order, axis=1)                 # [N, K]
    mx = topv.max(axis=1, keepdims=True)
    eg = np.exp(topv - mx)
    gates = eg / eg.sum(axis=1, keepdims=True)                       # [N, K]

    lmx = logits.max(axis=1, keepdims=True)
    lse = np.log(np.exp(logits - lmx).sum(axis=1, keepdims=True)) + lmx
    log_probs = logits - lse
    ideal = 1.0 / E
    aux = AUX_COEFF * np.mean(ideal * (np.log(ideal) - log_probs), dtype=np.float64)
    return order, gates, np.float32(aux)


def kernel(x, Wg, bg, W1, b1, W2, b2):
    from concourse.bass_utils import run_bass_kernel_spmd

    x = np.asarray(x)
    x_flat = np.ascontiguousarray(x.reshape(N, D), dtype=np.float32)
    Wg, bg = np.asarray(Wg), np.asarray(bg)
    W1, b1, W2, b2 = (np.asarray(a, dtype=np.float32) for a in (W1, b1, W2, b2))

    order, gates, aux = _route(x_flat, Wg, bg)

    # per-expert token index lists and combined gate weights
    idx_e, g_e = [], []
    for e in range(E):
        hits = order == e                           # [N, K]
        w = (gates * hits).sum(axis=1).astype(np.float32)
        idx = np.nonzero(hits.any(axis=1))[0]
        idx_e.append(idx)
        g_e.append(w[idx])
    max_cnt = max(len(i) for i in idx_e)
    rounds = max(1, -(-max_cnt // C))

    # static per-core tensors (weights), laid out for contiguous DMA
    x_bf = x_flat.astype(BF16)
    static_maps = []
    for e in range(E):
        w1h = np.ascontiguousarray(
            W1[e].astype(BF16).reshape(KD, P, MH, P).transpose(2, 1, 0, 3)
        )                                            # [MH, P, KD, P]
        w2h = np.ascontiguousarray(
            W2[e].astype(BF16).reshape(KH, P, DOUT).transpose(1, 0, 2)
        )                                            # [P, KH, DOUT]
        b1h = np.ascontiguousarray(b1[e].reshape(MH, P).T)           # [P, MH]
        b2h = np.ascontiguousarray(np.broadcast_to(b2[e], (P, DOUT)))  # [P, DOUT]
        static_maps.append({"w1": w1h, "w2": w2h, "b1v": b1h, "b2v": b2h})

    nc = _get_program()

    out_flat = np.zeros((N, DOUT), dtype=np.float32)
    for r in range(rounds):
        in_maps = []
        chunk_idx = []
        for e in range(E):
            idx = idx_e[e][r * C : (r + 1) * C]
            g = g_e[e][r * C : (r + 1) * C]
            chunk_idx.append(idx)
            xg = np.zeros((C, D), dtype=BF16)
            xg[: len(idx)] = x_bf[idx]
            xth = np.ascontiguousarray(
                xg.reshape(C, KD, P).transpose(2, 1, 0)
            )                                        # [P, KD, C]
            gp = np.zeros(C_PAD, dtype=np.float32)
            gp[: len(g)] = g
            gh = np.ascontiguousarray(gp.reshape(CT, P).T)           # [P, CT]
            in_maps.append({**static_maps[e], "xt": xth, "gv": gh})

        import os

        global LAST_EXEC_NS, LAST_TRACE
        trace = bool(int(os.environ.get("KERNEL_TRACE", "0")))
        res = run_bass_kernel_spmd(nc, in_maps, list(range(E)), trace=trace)
        if trace:
            LAST_EXEC_NS = res.exec_time_ns
            if res.instructions_and_trace is not None:
                LAST_TRACE = res.instructions_and_trace[1]
        results = res.results

        for e in range(E):
            idx = chunk_idx[e]
            y = np.asarray(results[e]["y"]).reshape(C_PAD, DOUT)
            # idx is unique within an expert, so fancy-index += is safe
            out_flat[idx] += y[: len(idx)]

    return out_flat.reshape(B, S, DOUT), aux


# revision 15
# speedup vs baseline: 1.0980x; 1.0899x over previous
"""MoE layer (top-2 of 8 experts) on 8 Trainium2 NeuronCores.

Strategy (expert-parallel, matching the sharding hint):
  - Host computes the gate (logits -> top-2 -> softmax) and the aux loss;
    this is 0.05% of the FLOPs.
  - Tokens are dispatched per expert on the host (the "all-to-all"), padded
    to a fixed capacity C, and core e runs expert e's FFN:
        y = gate * (GELU(x @ W1[e] + b1[e]) @ W2[e] + b2[e])
    on its block of routed tokens.  Both GEMMs run in bf16 with fp32 PSUM
    accumulation.  The first GEMM is computed transposed (A^T = W1^T @ x^T)
    so the second GEMM needs no on-device transpose.
  - Host scatter-adds the per-expert results back into token order.

Device time is dominated by the two GEMMs: 2 * C * (D*H + H*DOUT) flops
per core ~ 19 GFLOP, near the bf16 PE roofline.
"""

import numpy as np
import ml_dtypes

# ---- problem constants (hardcoded per contract) ----
B, S, D, DOUT, E, H, K = 2, 2048, 1024, 1024, 8, 4096, 2
N = B * S
AUX_COEFF = 0.01
P = 128
C = 1088            # per-expert token capacity per round (max count is 1066)
C_PAD = 1152        # C rounded up to a multiple of 128 (DRAM layout granularity)
KD = D // P         # 8  contraction chunks, GEMM1
MH = H // P         # 32 output row tiles, GEMM1 (H on partitions)
KH = H // P         # 32 contraction chunks, GEMM2
CT = C_PAD // P     # 9  output row tiles, GEMM2 (tokens on partitions)
C_TILES = [(0, 512), (512, 512), (1024, C - 1024)]   # GEMM1 free-dim tiling
M_TILES = [(i * P, min(P, C - i * P)) for i in range(CT) if i * P < C]

BF16 = ml_dtypes.bfloat16

_COMPILED = None  # cached (nc, run) so repeated kernel() calls reuse the NEFF
LAST_EXEC_NS = None   # max-core HW exec time from the last traced run
LAST_TRACE = None     # path to the last perfetto trace (trace runs only)


def _build_program(act="gelu"):
    import concourse.bass as bass
    import concourse.mybir as mybir
    import concourse.tile as tile
    from concourse import bacc

    act_fn = {
        "gelu": mybir.ActivationFunctionType.Gelu,
        # CoreSim has no Gelu LUT; identity lets the sim validate everything else
        "identity": mybir.ActivationFunctionType.Identity,
    }[act]

    f32 = mybir.dt.float32
    bf16 = mybir.dt.bfloat16

    nc = bacc.Bacc(
        "TRN2",
        target_bir_lowering=False,
        debug=False,
        enable_asserts=True,
        num_devices=8,
    )

    # Inputs are pre-laid-out on the host so every DMA is contiguous:
    #   xt : [P, KD, C]  bf16   xt[p,k,c]  = x_routed[c, k*P+p]
    #   w1 : [MH, P, KD, P] bf16 w1[m,p,k,j] = W1[k*P+p, m*P+j]
    #   w2 : [P, KH, DOUT] bf16 w2[p,k,n]  = W2[k*P+p, n]
    #   b1 : [P, MH] f32        b1[p,m]    = b1[m*P+p]
    #   b2 : [P, DOUT] f32      row-broadcast copy of b2
    #   gv : [P, CT] f32        gv[p,t]    = gate[t*P+p]
    xt_d = nc.dram_tensor("xt", [P, KD, C], bf16, kind="ExternalInput").ap()
    w1_d = nc.dram_tensor("w1", [MH, P, KD, P], bf16, kind="ExternalInput").ap()
    w2_d = nc.dram_tensor("w2", [P, KH, DOUT], bf16, kind="ExternalInput").ap()
    b1_d = nc.dram_tensor("b1v", [P, MH], f32, kind="ExternalInput").ap()
    b2_d = nc.dram_tensor("b2v", [P, DOUT], f32, kind="ExternalInput").ap()
    g_d = nc.dram_tensor("gv", [P, CT], f32, kind="ExternalInput").ap()
    y_d = nc.dram_tensor("y", [CT, P, DOUT], f32, kind="ExternalOutput").ap()

    with tile.TileContext(nc) as tc:
        with (
            tc.tile_pool(name="resident", bufs=1) as res,
            tc.tile_pool(name="w1s", bufs=3) as w1s,
            tc.tile_pool(name="yout", bufs=4) as yout,
            tc.tile_pool(name="ps", bufs=8, space="PSUM") as psp,
        ):
            # xt loaded per k-chunk so the first matmul starts after ~1 chunk;
            # w2/b2 go on the (otherwise idle) GpSimd DMA queue so the big W2
            # transfer doesn't block the W1 stream on the Sync queue.
            xt_sb = res.tile([P, KD, C], bf16)
            for k in range(KD):
                nc.sync.dma_start(xt_sb[:, k], xt_d[:, k])
            b1_sb = res.tile([P, MH], f32)
            nc.sync.dma_start(b1_sb[:], b1_d[:])
            g_sb = res.tile([P, CT], f32)
            nc.sync.dma_start(g_sb[:], g_d[:])
            w2_sb = res.tile([P, KH, DOUT], bf16)
            b2_sb = res.tile([P, DOUT], f32)
            hmt_sb = res.tile([P, KH, C], bf16)

            # GEMM1: hmt[:, m, :] = GELU(W1_m^T @ x^T + b1_m), H on partitions
            for m in range(MH):
                w1_t = w1s.tile([P, KD, P], bf16, tag="w1t")
                nc.sync.dma_start(w1_t[:], w1_d[m])
                # stream one W2 chunk per GEMM1 m-tile: keeps the startup
                # DMA path short (xt + first w1 only) while W2 fully lands
                # before GEMM2 begins
                nc.sync.dma_start(w2_sb[:, m], w2_d[:, m])
                if m == 0:
                    nc.sync.dma_start(b2_sb[:], b2_d[:])
                ps_tiles = []
                for noff, nsz in C_TILES:
                    ps_tiles.append(psp.tile([P, nsz], f32, tag="ps", name=f"ps1_{m}_{noff}"))
                for k in range(KD):
                    for (noff, nsz), ps in zip(C_TILES, ps_tiles):
                        nc.tensor.matmul(
                            ps[:],
                            lhsT=w1_t[:, k],
                            rhs=xt_sb[:, k, noff : noff + nsz],
                            start=(k == 0),
                            stop=(k == KD - 1),
                        )
                for (noff, nsz), ps in zip(C_TILES, ps_tiles):
                    nc.scalar.activation(
                        hmt_sb[:, m, noff : noff + nsz],
                        ps[:],
                        act_fn,
                        bias=b1_sb[:, m : m + 1],
                        scale=1.0,
                    )

            # GEMM2: y[mc] = gate * (Hm_mc @ W2 + b2), tokens on partitions
            for mc, (roff, rows) in enumerate(M_TILES):
                ps0 = psp.tile([P, 512], f32, tag="ps", name=f"ps2a_{mc}")
                ps1 = psp.tile([P, 512], f32, tag="ps", name=f"ps2b_{mc}")
                for k in range(KH):
                    lhs = hmt_sb[:, k, roff : roff + rows]
                    nc.tensor.matmul(
                        ps0[:rows], lhsT=lhs, rhs=w2_sb[:, k, 0:512],
                        start=(k == 0), stop=(k == KH - 1),
                    )
                    nc.tensor.matmul(
                        ps1[:rows], lhsT=lhs, rhs=w2_sb[:, k, 512:1024],
                        start=(k == 0), stop=(k == KH - 1),
                    )
                y_t = yout.tile([P, DOUT], f32, tag="yt")
                nc.vector.tensor_add(y_t[:rows, 0:512], ps0[:rows], b2_sb[:rows, 0:512])
                nc.vector.tensor_add(y_t[:rows, 512:1024], ps1[:rows], b2_sb[:rows, 512:1024])
                nc.scalar.mul(y_t[:rows], y_t[:rows], g_sb[:rows, mc : mc + 1])
                nc.sync.dma_start(y_d[mc, :rows], y_t[:rows])

    nc.compile()
    return nc


def _get_program():
    global _COMPILED
    if _COMPILED is None:
        _COMPILED = _build_program()
    return _COMPILED


def _route(x_flat, Wg, bg):
    """Host gating: logits, top-2 (matches jax.lax.top_k tie-breaking),
    softmax gates, aux loss."""
    logits = x_flat.astype(np.float32) @ Wg.astype(np.float32) + bg.astype(np.float32)
    order = np.argsort(-logits, axis=-1, kind="stable")[:, :K]       # [N, K]
    topv = np.take_along_axis(logits, order, axis=1)                 # [N, K]
    mx = topv.max(axis=1, keepdims=True)
    eg = np.exp(topv - mx)
    gates = eg / eg.sum(axis=1, keepdims=True)                       # [N, K]

    lmx = logits.max(axis=1, keepdims=True)
    lse = np.log(np.exp(logits - lmx).sum(axis=1, keepdims=True)) + lmx
    log_probs = logits - lse
    ideal = 1.0 / E
    aux = AUX_COEFF * np.mean(ideal * (np.log(ideal) - log_probs), dtype=np.float64)
    return order, gates, np.float32(aux)


def kernel(x, Wg, bg, W1, b1, W2, b2):
    from concourse.bass_utils import run_bass_kernel_spmd

    x = np.asarray(x)
    x_flat = np.ascontiguousarray(x.reshape(N, D), dtype=np.float32)
    Wg, bg = np.asarray(Wg), np.asarray(bg)
    W1, b1, W2, b2 = (np.asarray(a, dtype=np.float32) for a in (W1, b1, W2, b2))

    order, gates, aux = _route(x_flat, Wg, bg)

    # per-expert token index lists and combined gate weights
    idx_e, g_e = [], []
    for e in range(E):
        hits = order == e                           # [N, K]
        w = (gates * hits).sum(axis=1).astype(np.float32)
        idx = np.nonzero(hits.any(axis=1))[0]
        idx_e.append(idx)
        g_e.append(w[idx])
    max_cnt = max(len(i) for i in idx_e)
    rounds = max(1, -(-max_cnt // C))

    # static per-core tensors (weights), laid out for contiguous DMA
    x_bf = x_flat.astype(BF16)
    static_maps = []
    for e in range(E):
        w1h = np.ascontiguousarray(
            W1[e].astype(BF16).reshape(KD, P, MH, P).transpose(2, 1, 0, 3)
        )                                            # [MH, P, KD, P]
        w2h = np.ascontiguousarray(
            W2[e].astype(BF16).reshape(KH, P, DOUT).transpose(1, 0, 2)
        )                                            # [P, KH, DOUT]
        b1h = np.ascontiguousarray(b1[e].reshape(MH, P).T)           # [P, MH]
        b2h = np.ascontiguousarray(np.broadcast_to(b2[e], (P, DOUT)))  # [P, DOUT]
        static_maps.append({"w1": w1h, "w2": w2h, "b1v": b1h, "b2v": b2h})

    nc = _get_program()

    out_flat = np.zeros((N, DOUT), dtype=np.float32)
    for r in range(rounds):
        in_maps = []
        chunk_idx = []
        for e in range(E):
            idx = idx_e[e][r * C : (r + 1) * C]
            g = g_e[e][r * C : (r + 1) * C]
            chunk_idx.append(idx)
            xg = np.zeros((C, D), dtype=BF16)
            xg[: len(idx)] = x_bf[idx]
            xth = np.ascontiguousarray(
                xg.reshape(C, KD, P).transpose(2, 1, 0)
            )                                        # [P, KD, C]
            gp = np.zeros(C_PAD, dtype=np.float32)
            gp[: len(g)] = g
            gh = np.ascontiguousarray(gp.reshape(CT, P).T)           # [P, CT]
            in_maps.append({**static_maps[e], "xt": xth, "gv": gh})

        import os

        global LAST_EXEC_NS, LAST_TRACE
        trace = bool(int(os.environ.get("KERNEL_TRACE", "0")))
        res = run_bass_kernel_spmd(nc, in_maps, list(range(E)), trace=trace)
        if trace:
            LAST_EXEC_NS = res.exec_time_ns
            if res.instructions_and_trace is not None:
                LAST_TRACE = res.instructions_and_trace[1]
        results = res.results

        for e in range(E):
            idx = chunk_idx[e]
            y = np.asarray(results[e]["y"]).reshape(C_PAD, DOUT)
            # idx is unique within an expert, so fancy-index += is safe
            out_flat[idx] += y[: len(idx)]

    return out_flat.reshape(B, S, DOUT), aux


# revision 20
# speedup vs baseline: 1.1234x; 1.0232x over previous
"""MoE layer (top-2 of 8 experts) on 8 Trainium2 NeuronCores.

Strategy (expert-parallel, matching the sharding hint):
  - Host computes the gate (logits -> top-2 -> softmax) and the aux loss;
    this is 0.05% of the FLOPs.
  - Tokens are dispatched per expert on the host (the "all-to-all"), padded
    to a fixed capacity C, and core e runs expert e's FFN:
        y = gate * (GELU(x @ W1[e] + b1[e]) @ W2[e] + b2[e])
    on its block of routed tokens.  Both GEMMs run in bf16 with fp32 PSUM
    accumulation.  The first GEMM is computed transposed (A^T = W1^T @ x^T)
    so the second GEMM needs no on-device transpose.
  - Host scatter-adds the per-expert results back into token order.

Device time is dominated by the two GEMMs: 2 * C * (D*H + H*DOUT) flops
per core ~ 19 GFLOP, near the bf16 PE roofline.
"""

import numpy as np
import ml_dtypes

# ---- problem constants (hardcoded per contract) ----
B, S, D, DOUT, E, H, K = 2, 2048, 1024, 1024, 8, 4096, 2
N = B * S
AUX_COEFF = 0.01
P = 128
C = 1088            # per-expert token capacity per round (max count is 1066)
C_PAD = 1152        # C rounded up to a multiple of 128 (DRAM layout granularity)
KD = D // P         # 8  contraction chunks, GEMM1
MH = H // P         # 32 output row tiles, GEMM1 (H on partitions)
KH = H // P         # 32 contraction chunks, GEMM2
CT = C_PAD // P     # 9  output row tiles, GEMM2 (tokens on partitions)
C_TILES = [(0, 512), (512, 512), (1024, C - 1024)]   # GEMM1 free-dim tiling
M_TILES = [(i * P, min(P, C - i * P)) for i in range(CT) if i * P < C]

BF16 = ml_dtypes.bfloat16

_COMPILED = None  # cached (nc, run) so repeated kernel() calls reuse the NEFF
LAST_EXEC_NS = None   # max-core HW exec time from the last traced run
LAST_TRACE = None     # path to the last perfetto trace (trace runs only)


def _build_program(act="gelu"):
    import concourse.bass as bass
    import concourse.mybir as mybir
    import concourse.tile as tile
    from concourse import bacc

    act_fn = {
        "gelu": mybir.ActivationFunctionType.Gelu,
        # CoreSim has no Gelu LUT; identity lets the sim validate everything else
        "identity": mybir.ActivationFunctionType.Identity,
    }[act]

    f32 = mybir.dt.float32
    bf16 = mybir.dt.bfloat16

    nc = bacc.Bacc(
        "TRN2",
        target_bir_lowering=False,
        debug=False,
        enable_asserts=True,
        num_devices=8,
    )

    # Inputs are pre-laid-out on the host so every DMA is contiguous:
    #   xt : [P, KD, C]  bf16   xt[p,k,c]  = x_routed[c, k*P+p]
    #   w1 : [MH, P, KD, P] bf16 w1[m,p,k,j] = W1[k*P+p, m*P+j]
    #   w2 : [P, KH, DOUT] bf16 w2[p,k,n]  = W2[k*P+p, n]
    #   b1 : [P, MH] f32        b1[p,m]    = b1[m*P+p]
    #   b2 : [P, DOUT] f32      row-broadcast copy of b2
    #   gv : [P, CT] f32        gv[p,t]    = gate[t*P+p]
    xt_d = nc.dram_tensor("xt", [P, KD, C], bf16, kind="ExternalInput").ap()
    w1_d = nc.dram_tensor("w1", [MH, P, KD, P], bf16, kind="ExternalInput").ap()
    w2_d = nc.dram_tensor("w2", [P, KH, DOUT], bf16, kind="ExternalInput").ap()
    b1_d = nc.dram_tensor("b1v", [P, MH], f32, kind="ExternalInput").ap()
    g_d = nc.dram_tensor("gv", [P, CT], f32, kind="ExternalInput").ap()
    y_d = nc.dram_tensor("y", [CT, P, DOUT], f32, kind="ExternalOutput").ap()

    with tile.TileContext(nc) as tc:
        with (
            tc.tile_pool(name="resident", bufs=1) as res,
            tc.tile_pool(name="w1s", bufs=3) as w1s,
            tc.tile_pool(name="yout", bufs=4) as yout,
            tc.tile_pool(name="ps", bufs=8, space="PSUM") as psp,
        ):
            # xt loaded per k-chunk so the first matmul starts after ~1 chunk;
            # w2/b2 go on the (otherwise idle) GpSimd DMA queue so the big W2
            # transfer doesn't block the W1 stream on the Sync queue.
            # First matmul needs only xt chunk 0 + w1 tile 0 — issue those
            # first so compute starts ~10us in instead of waiting for the
            # full activation load.
            xt_sb = res.tile([P, KD, C], bf16)
            nc.sync.dma_start(xt_sb[:, 0], xt_d[:, 0])
            w1_t0 = w1s.tile([P, KD, P], bf16, tag="w1t", name="w1_t0")
            nc.sync.dma_start(w1_t0[:], w1_d[0])
            for k in range(1, KD):
                nc.sync.dma_start(xt_sb[:, k], xt_d[:, k])
            b1_sb = res.tile([P, MH], f32)
            nc.sync.dma_start(b1_sb[:], b1_d[:])
            g_sb = res.tile([P, CT], f32)
            nc.sync.dma_start(g_sb[:], g_d[:])
            w2_sb = res.tile([P, KH, DOUT], bf16)
            hmt_sb = res.tile([P, KH, C], bf16)

            # GEMM1: hmt[:, m, :] = GELU(W1_m^T @ x^T + b1_m), H on partitions
            for m in range(MH):
                if m == 0:
                    w1_t = w1_t0
                else:
                    w1_t = w1s.tile([P, KD, P], bf16, tag="w1t")
                    nc.sync.dma_start(w1_t[:], w1_d[m])
                # stream one W2 chunk per GEMM1 m-tile: keeps the startup
                # DMA path short while W2 fully lands before GEMM2 begins
                nc.sync.dma_start(w2_sb[:, m], w2_d[:, m])
                ps_tiles = []
                for noff, nsz in C_TILES:
                    ps_tiles.append(psp.tile([P, nsz], f32, tag="ps", name=f"ps1_{m}_{noff}"))
                for k in range(KD):
                    for (noff, nsz), ps in zip(C_TILES, ps_tiles):
                        nc.tensor.matmul(
                            ps[:],
                            lhsT=w1_t[:, k],
                            rhs=xt_sb[:, k, noff : noff + nsz],
                            start=(k == 0),
                            stop=(k == KD - 1),
                        )
                for (noff, nsz), ps in zip(C_TILES, ps_tiles):
                    nc.scalar.activation(
                        hmt_sb[:, m, noff : noff + nsz],
                        ps[:],
                        act_fn,
                        bias=b1_sb[:, m : m + 1],
                        scale=1.0,
                    )

            # GEMM2: y[mc] = gate * (Hm_mc @ W2 + b2), tokens on partitions
            for mc, (roff, rows) in enumerate(M_TILES):
                ps0 = psp.tile([P, 512], f32, tag="ps", name=f"ps2a_{mc}")
                ps1 = psp.tile([P, 512], f32, tag="ps", name=f"ps2b_{mc}")
                for k in range(KH):
                    lhs = hmt_sb[:, k, roff : roff + rows]
                    nc.tensor.matmul(
                        ps0[:rows], lhsT=lhs, rhs=w2_sb[:, k, 0:512],
                        start=(k == 0), stop=(k == KH - 1),
                    )
                    nc.tensor.matmul(
                        ps1[:rows], lhsT=lhs, rhs=w2_sb[:, k, 512:1024],
                        start=(k == 0), stop=(k == KH - 1),
                    )
                # b2 is added on the host (comb_w @ b2); device applies only
                # the gate scale, split across DVE and ACT so both halves
                # run in parallel.
                y_t = yout.tile([P, DOUT], f32, tag="yt")
                nc.vector.tensor_scalar_mul(
                    y_t[:rows, 0:512], ps0[:rows], g_sb[:rows, mc : mc + 1]
                )
                nc.scalar.mul(y_t[:rows, 512:1024], ps1[:rows], g_sb[:rows, mc : mc + 1])
                nc.sync.dma_start(y_d[mc, :rows], y_t[:rows])

    nc.compile()
    return nc


def _get_program():
    global _COMPILED
    if _COMPILED is None:
        _COMPILED = _build_program()
    return _COMPILED


def _route(x_flat, Wg, bg):
    """Host gating: logits, top-2 (matches jax.lax.top_k tie-breaking),
    softmax gates, aux loss."""
    logits = x_flat.astype(np.float32) @ Wg.astype(np.float32) + bg.astype(np.float32)
    order = np.argsort(-logits, axis=-1, kind="stable")[:, :K]       # [N, K]
    topv = np.take_along_axis(logits, order, axis=1)                 # [N, K]
    mx = topv.max(axis=1, keepdims=True)
    eg = np.exp(topv - mx)
    gates = eg / eg.sum(axis=1, keepdims=True)                       # [N, K]

    lmx = logits.max(axis=1, keepdims=True)
    lse = np.log(np.exp(logits - lmx).sum(axis=1, keepdims=True)) + lmx
    log_probs = logits - lse
    ideal = 1.0 / E
    aux = AUX_COEFF * np.mean(ideal * (np.log(ideal) - log_probs), dtype=np.float64)
    return order, gates, np.float32(aux)


def kernel(x, Wg, bg, W1, b1, W2, b2):
    from concourse.bass_utils import run_bass_kernel_spmd

    x = np.asarray(x)
    x_flat = np.ascontiguousarray(x.reshape(N, D), dtype=np.float32)
    Wg, bg = np.asarray(Wg), np.asarray(bg)
    W1, b1, W2, b2 = (np.asarray(a, dtype=np.float32) for a in (W1, b1, W2, b2))

    order, gates, aux = _route(x_flat, Wg, bg)

    # per-expert token index lists and combined gate weights
    idx_e, g_e = [], []
    for e in range(E):
        hits = order == e                           # [N, K]
        w = (gates * hits).sum(axis=1).astype(np.float32)
        idx = np.nonzero(hits.any(axis=1))[0]
        idx_e.append(idx)
        g_e.append(w[idx])
    max_cnt = max(len(i) for i in idx_e)
    rounds = max(1, -(-max_cnt // C))

    # static per-core tensors (weights), laid out for contiguous DMA
    x_bf = x_flat.astype(BF16)
    static_maps = []
    for e in range(E):
        w1h = np.ascontiguousarray(
            W1[e].astype(BF16).reshape(KD, P, MH, P).transpose(2, 1, 0, 3)
        )                                            # [MH, P, KD, P]
        w2h = np.ascontiguousarray(
            W2[e].astype(BF16).reshape(KH, P, DOUT).transpose(1, 0, 2)
        )                                            # [P, KH, DOUT]
        b1h = np.ascontiguousarray(b1[e].reshape(MH, P).T)           # [P, MH]
        static_maps.append({"w1": w1h, "w2": w2h, "b1v": b1h})

    nc = _get_program()

    # b2 contribution, applied on the host: out[n] += sum_k gate[n,k]*b2[e_nk]
    out_flat = gates[:, 0, None] * b2[order[:, 0]] + gates[:, 1, None] * b2[order[:, 1]]
    out_flat = out_flat.astype(np.float32)
    for r in range(rounds):
        in_maps = []
        chunk_idx = []
        for e in range(E):
            idx = idx_e[e][r * C : (r + 1) * C]
            g = g_e[e][r * C : (r + 1) * C]
            chunk_idx.append(idx)
            xg = np.zeros((C, D), dtype=BF16)
            xg[: len(idx)] = x_bf[idx]
            xth = np.ascontiguousarray(
                xg.reshape(C, KD, P).transpose(2, 1, 0)
            )                                        # [P, KD, C]
            gp = np.zeros(C_PAD, dtype=np.float32)
            gp[: len(g)] = g
            gh = np.ascontiguousarray(gp.reshape(CT, P).T)           # [P, CT]
            in_maps.append({**static_maps[e], "xt": xth, "gv": gh})

        import os

        global LAST_EXEC_NS, LAST_TRACE
        trace = bool(int(os.environ.get("KERNEL_TRACE", "0")))
        res = run_bass_kernel_spmd(nc, in_maps, list(range(E)), trace=trace)
        if trace:
            LAST_EXEC_NS = res.exec_time_ns
            if res.instructions_and_trace is not None:
                LAST_TRACE = res.instructions_and_trace[1]
        results = res.results

        for e in range(E):
            idx = chunk_idx[e]
            y = np.asarray(results[e]["y"]).reshape(C_PAD, DOUT)
            # idx is unique within an expert, so fancy-index += is safe
            out_flat[idx] += y[: len(idx)]

    return out_flat.reshape(B, S, DOUT), aux


# revision 22
# speedup vs baseline: 1.1262x; 1.0024x over previous
"""MoE layer (top-2 of 8 experts) on 8 Trainium2 NeuronCores.

Strategy (expert-parallel, matching the sharding hint):
  - Host computes the gate (logits -> top-2 -> softmax) and the aux loss;
    this is 0.05% of the FLOPs.
  - Tokens are dispatched per expert on the host (the "all-to-all"), padded
    to a fixed capacity C, and core e runs expert e's FFN:
        y = gate * (GELU(x @ W1[e] + b1[e]) @ W2[e] + b2[e])
    on its block of routed tokens.  Both GEMMs run in bf16 with fp32 PSUM
    accumulation.  The first GEMM is computed transposed (A^T = W1^T @ x^T)
    so the second GEMM needs no on-device transpose.
  - Host scatter-adds the per-expert results back into token order.

Device time is dominated by the two GEMMs: 2 * C * (D*H + H*DOUT) flops
per core ~ 19 GFLOP, near the bf16 PE roofline.
"""

import numpy as np
import ml_dtypes

# ---- problem constants (hardcoded per contract) ----
B, S, D, DOUT, E, H, K = 2, 2048, 1024, 1024, 8, 4096, 2
N = B * S
AUX_COEFF = 0.01
P = 128
C = 1088            # per-expert token capacity per round (max count is 1066)
C_PAD = 1152        # C rounded up to a multiple of 128 (DRAM layout granularity)
KD = D // P         # 8  contraction chunks, GEMM1
MH = H // P         # 32 output row tiles, GEMM1 (H on partitions)
KH = H // P         # 32 contraction chunks, GEMM2
CT = C_PAD // P     # 9  output row tiles, GEMM2 (tokens on partitions)
C_TILES = [(0, 512), (512, 512), (1024, C - 1024)]   # GEMM1 free-dim tiling
M_TILES = [(i * P, min(P, C - i * P)) for i in range(CT) if i * P < C]

BF16 = ml_dtypes.bfloat16

_COMPILED = None  # cached (nc, run) so repeated kernel() calls reuse the NEFF
LAST_EXEC_NS = None   # max-core HW exec time from the last traced run
LAST_TRACE = None     # path to the last perfetto trace (trace runs only)


def _build_program(act="gelu"):
    import concourse.bass as bass
    import concourse.mybir as mybir
    import concourse.tile as tile
    from concourse import bacc

    act_fn = {
        "gelu": mybir.ActivationFunctionType.Gelu,
        # CoreSim has no Gelu LUT; identity lets the sim validate everything else
        "identity": mybir.ActivationFunctionType.Identity,
    }[act]

    f32 = mybir.dt.float32
    bf16 = mybir.dt.bfloat16

    nc = bacc.Bacc(
        "TRN2",
        target_bir_lowering=False,
        debug=False,
        enable_asserts=True,
        num_devices=8,
    )

    # Inputs are pre-laid-out on the host so every DMA is contiguous:
    #   xt : [P, KD, C]  bf16   xt[p,k,c]  = x_routed[c, k*P+p]
    #   w1 : [MH, P, KD, P] bf16 w1[m,p,k,j] = W1[k*P+p, m*P+j]
    #   w2 : [P, KH, DOUT] bf16 w2[p,k,n]  = W2[k*P+p, n]
    #   b1 : [P, MH] f32        b1[p,m]    = b1[m*P+p]
    #   b2 : [P, DOUT] f32      row-broadcast copy of b2
    #   gv : [P, CT] f32        gv[p,t]    = gate[t*P+p]
    xt_d = nc.dram_tensor("xt", [P, KD, C], bf16, kind="ExternalInput").ap()
    w1_d = nc.dram_tensor("w1", [MH, P, KD, P], bf16, kind="ExternalInput").ap()
    w2_d = nc.dram_tensor("w2", [P, KH, DOUT], bf16, kind="ExternalInput").ap()
    b1_d = nc.dram_tensor("b1v", [P, MH], f32, kind="ExternalInput").ap()
    g_d = nc.dram_tensor("gv", [P, CT], f32, kind="ExternalInput").ap()
    y_d = nc.dram_tensor("y", [CT, P, DOUT], f32, kind="ExternalOutput").ap()

    with tile.TileContext(nc) as tc:
        with (
            tc.tile_pool(name="resident", bufs=1) as res,
            tc.tile_pool(name="w1s", bufs=3) as w1s,
            tc.tile_pool(name="yout", bufs=4) as yout,
            tc.tile_pool(name="ps", bufs=8, space="PSUM") as psp,
        ):
            # xt loaded per k-chunk so the first matmul starts after ~1 chunk;
            # w2/b2 go on the (otherwise idle) GpSimd DMA queue so the big W2
            # transfer doesn't block the W1 stream on the Sync queue.
            # First matmul needs only xt chunk 0 + w1 tile 0 — issue those
            # first so compute starts ~10us in instead of waiting for the
            # full activation load.
            xt_sb = res.tile([P, KD, C], bf16)
            nc.sync.dma_start(xt_sb[:, 0], xt_d[:, 0])
            w1_t0 = w1s.tile([P, KD, P], bf16, tag="w1t", name="w1_t0")
            nc.sync.dma_start(w1_t0[:], w1_d[0])
            for k in range(1, KD):
                nc.sync.dma_start(xt_sb[:, k], xt_d[:, k])
            b1_sb = res.tile([P, MH], f32)
            nc.sync.dma_start(b1_sb[:], b1_d[:])
            g_sb = res.tile([P, CT], f32)
            nc.sync.dma_start(g_sb[:], g_d[:])
            w2_sb = res.tile([P, KH, DOUT], bf16)
            hmt_sb = res.tile([P, KH, C], bf16)

            # PE warmup: the HAM clock gate holds the PE at 1.2 GHz until it
            # has been busy ~3.4us.  Burn that window on scratch matmuls while
            # the input DMAs stream, so real matmuls start at 2.4 GHz.
            warm_sb = res.tile([P, P], bf16)
            nc.any.memset(warm_sb[:], 0.0)
            warm_ps = psp.tile([P, P], f32, tag="ps", name="warm_ps")
            for i in range(40):
                nc.tensor.matmul(
                    warm_ps[:], lhsT=warm_sb[:], rhs=warm_sb[:],
                    start=(i == 0), stop=(i == 39),
                )

            # GEMM1: hmt[:, m, :] = GELU(W1_m^T @ x^T + b1_m), H on partitions
            for m in range(MH):
                if m == 0:
                    w1_t = w1_t0
                else:
                    w1_t = w1s.tile([P, KD, P], bf16, tag="w1t")
                    nc.sync.dma_start(w1_t[:], w1_d[m])
                # stream one W2 chunk per GEMM1 m-tile: keeps the startup
                # DMA path short while W2 fully lands before GEMM2 begins
                nc.sync.dma_start(w2_sb[:, m], w2_d[:, m])
                ps_tiles = []
                for noff, nsz in C_TILES:
                    ps_tiles.append(psp.tile([P, nsz], f32, tag="ps", name=f"ps1_{m}_{noff}"))
                for k in range(KD):
                    for (noff, nsz), ps in zip(C_TILES, ps_tiles):
                        nc.tensor.matmul(
                            ps[:],
                            lhsT=w1_t[:, k],
                            rhs=xt_sb[:, k, noff : noff + nsz],
                            start=(k == 0),
                            stop=(k == KD - 1),
                        )
                for (noff, nsz), ps in zip(C_TILES, ps_tiles):
                    nc.scalar.activation(
                        hmt_sb[:, m, noff : noff + nsz],
                        ps[:],
                        act_fn,
                        bias=b1_sb[:, m : m + 1],
                        scale=1.0,
                    )

            # GEMM2: y[mc] = gate * (Hm_mc @ W2 + b2), tokens on partitions
            for mc, (roff, rows) in enumerate(M_TILES):
                ps0 = psp.tile([P, 512], f32, tag="ps", name=f"ps2a_{mc}")
                ps1 = psp.tile([P, 512], f32, tag="ps", name=f"ps2b_{mc}")
                for k in range(KH):
                    lhs = hmt_sb[:, k, roff : roff + rows]
                    nc.tensor.matmul(
                        ps0[:rows], lhsT=lhs, rhs=w2_sb[:, k, 0:512],
                        start=(k == 0), stop=(k == KH - 1),
                    )
                    nc.tensor.matmul(
                        ps1[:rows], lhsT=lhs, rhs=w2_sb[:, k, 512:1024],
                        start=(k == 0), stop=(k == KH - 1),
                    )
                # b2 is added on the host (comb_w @ b2); device applies only
                # the gate scale, split across DVE and ACT so both halves
                # run in parallel.
                y_t = yout.tile([P, DOUT], f32, tag="yt")
                nc.vector.tensor_scalar_mul(
                    y_t[:rows, 0:512], ps0[:rows], g_sb[:rows, mc : mc + 1]
                )
                nc.scalar.mul(y_t[:rows, 512:1024], ps1[:rows], g_sb[:rows, mc : mc + 1])
                nc.sync.dma_start(y_d[mc, :rows], y_t[:rows])

    nc.compile()
    return nc


def _get_program():
    global _COMPILED
    if _COMPILED is None:
        _COMPILED = _build_program()
    return _COMPILED


def _route(x_flat, Wg, bg):
    """Host gating: logits, top-2 (matches jax.lax.top_k tie-breaking),
    softmax gates, aux loss."""
    logits = x_flat.astype(np.float32) @ Wg.astype(np.float32) + bg.astype(np.float32)
    order = np.argsort(-logits, axis=-1, kind="stable")[:, :K]       # [N, K]
    topv = np.take_along_axis(logits, order, axis=1)                 # [N, K]
    mx = topv.max(axis=1, keepdims=True)
    eg = np.exp(topv - mx)
    gates = eg / eg.sum(axis=1, keepdims=True)                       # [N, K]

    lmx = logits.max(axis=1, keepdims=True)
    lse = np.log(np.exp(logits - lmx).sum(axis=1, keepdims=True)) + lmx
    log_probs = logits - lse
    ideal = 1.0 / E
    aux = AUX_COEFF * np.mean(ideal * (np.log(ideal) - log_probs), dtype=np.float64)
    return order, gates, np.float32(aux)


def kernel(x, Wg, bg, W1, b1, W2, b2):
    from concourse.bass_utils import run_bass_kernel_spmd

    x = np.asarray(x)
    x_flat = np.ascontiguousarray(x.reshape(N, D), dtype=np.float32)
    Wg, bg = np.asarray(Wg), np.asarray(bg)
    W1, b1, W2, b2 = (np.asarray(a, dtype=np.float32) for a in (W1, b1, W2, b2))

    order, gates, aux = _route(x_flat, Wg, bg)

    # per-expert token index lists and combined gate weights
    idx_e, g_e = [], []
    for e in range(E):
        hits = order == e                           # [N, K]
        w = (gates * hits).sum(axis=1).astype(np.float32)
        idx = np.nonzero(hits.any(axis=1))[0]
        idx_e.append(idx)
        g_e.append(w[idx])
    max_cnt = max(len(i) for i in idx_e)
    rounds = max(1, -(-max_cnt // C))

    # static per-core tensors (weights), laid out for contiguous DMA
    x_bf = x_flat.astype(BF16)
    static_maps = []
    for e in range(E):
        w1h = np.ascontiguousarray(
            W1[e].astype(BF16).reshape(KD, P, MH, P).transpose(2, 1, 0, 3)
        )                                            # [MH, P, KD, P]
        w2h = np.ascontiguousarray(
            W2[e].astype(BF16).reshape(KH, P, DOUT).transpose(1, 0, 2)
        )                                            # [P, KH, DOUT]
        b1h = np.ascontiguousarray(b1[e].reshape(MH, P).T)           # [P, MH]
        static_maps.append({"w1": w1h, "w2": w2h, "b1v": b1h})

    nc = _get_program()

    # b2 contribution, applied on the host: out[n] += sum_k gate[n,k]*b2[e_nk]
    out_flat = gates[:, 0, None] * b2[order[:, 0]] + gates[:, 1, None] * b2[order[:, 1]]
    out_flat = out_flat.astype(np.float32)
    for r in range(rounds):
        in_maps = []
        chunk_idx = []
        for e in range(E):
            idx = idx_e[e][r * C : (r + 1) * C]
            g = g_e[e][r * C : (r + 1) * C]
            chunk_idx.append(idx)
            xg = np.zeros((C, D), dtype=BF16)
            xg[: len(idx)] = x_bf[idx]
            xth = np.ascontiguousarray(
                xg.reshape(C, KD, P).transpose(2, 1, 0)
            )                                        # [P, KD, C]
            gp = np.zeros(C_PAD, dtype=np.float32)
            gp[: len(g)] = g
            gh = np.ascontiguousarray(gp.reshape(CT, P).T)           # [P, CT]
            in_maps.append({**static_maps[e], "xt": xth, "gv": gh})

        import os

        global LAST_EXEC_NS, LAST_TRACE
        trace = bool(int(os.environ.get("KERNEL_TRACE", "0")))
        res = run_bass_kernel_spmd(nc, in_maps, list(range(E)), trace=trace)
        if trace:
            LAST_EXEC_NS = res.exec_time_ns
            if res.instructions_and_trace is not None:
                LAST_TRACE = res.instructions_and_trace[1]
        results = res.results

        for e in range(E):
            idx = chunk_idx[e]
            y = np.asarray(results[e]["y"]).reshape(C_PAD, DOUT)
            # idx is unique within an expert, so fancy-index += is safe
            out_flat[idx] += y[: len(idx)]

    return out_flat.reshape(B, S, DOUT), aux
